# revision 14
# baseline (speedup 1.0000x reference)
"""Trainium2 Bass kernel for nn_CoSSM: 3x (Conv1d+BN+ReLU+skip -> Mamba -> LN residual).

Sharding: data-parallel over batch, 1 sample per NeuronCore (B=8, 8 cores).
Device layout: channels on partitions, sequence t on the free axis.

Mamba selective scan runs on VectorE via tensor_tensor_scan: state components
(d-blocks) are chained along the free axis with zeroed separator columns, one
scan instruction per state index n.
"""

import os
import sys
from contextlib import ExitStack

import numpy as np

for _p in ("/opt/trn_rl_repo",):
    if _p not in sys.path and os.path.isdir(_p):
        sys.path.insert(0, _p)

import ml_dtypes

import concourse.bass as bass
import concourse.tile as tile
from concourse import bacc, mybir
from concourse.alu_op_type import AluOpType
from concourse.bass_utils import run_bass_kernel_spmd

F32 = mybir.dt.float32
BF16 = mybir.dt.bfloat16
F32R = mybir.dt.float32r
AF = mybir.ActivationFunctionType
OP = AluOpType
NPBF16 = ml_dtypes.bfloat16

B, L = 8, 512
SEP = L + 1  # block stride along free axis (separator col + L timesteps)
N_CORES = 8
EPS_BN, EPS_LN = 1e-5, 1e-6

LAYERS = [
    dict(prev=256, d=256, din=512, R=16, skip=False),
    dict(prev=256, d=512, din=1024, R=32, skip=True),
    dict(prev=512, d=512, din=1024, R=32, skip=False),
]

_cache = {}
last_run_info = {}


# --------------------------------------------------------------------------
# host-side weight prep
# --------------------------------------------------------------------------
def _prep_weights(params):
    """Returns (tensors: dict name -> np.ndarray, a_ns: list of 16-float lists
    or None per layer)."""
    t = {}
    a_ns = []
    for l, (p, m) in enumerate(zip(params, LAYERS)):
        prev, d, din, R = m["prev"], m["d"], m["din"], m["R"]
        pk, pd, nb, ne = prev // 128, d // 128, din // 128, 2 * din // 128

        g = np.asarray(p["bn_gamma"], np.float32)
        v = np.asarray(p["bn_var"], np.float32)
        mu = np.asarray(p["bn_mean"], np.float32)
        be = np.asarray(p["bn_beta"], np.float32)
        s = g / np.sqrt(v + EPS_BN)
        wc = np.asarray(p["conv_w"], np.float32) * s[:, None, None]  # (d, prev, 3)
        bconv = be - mu * s

        wcT = np.transpose(wc, (1, 2, 0))  # (prev, 3, d)
        t[f"convw{l}"] = np.ascontiguousarray(
            wcT.reshape(pk, 128, 3, pd, 128).transpose(0, 2, 3, 1, 4)
        )  # (pk, 3, pd, 128, 128)
        if m["skip"]:
            sw = np.asarray(p["skip_w"], np.float32)[:, :, 0].T  # (prev, d)
            t[f"skipw{l}"] = np.ascontiguousarray(
                sw.reshape(pk, 128, pd, 128).transpose(0, 2, 1, 3)
            )  # (pk, pd, 128, 128)

        ipT = np.asarray(p["in_proj_w"], np.float32).T  # (d, 2din)
        t[f"inpw{l}"] = np.ascontiguousarray(
            ipT.reshape(pd, 128, ne, 128).transpose(0, 2, 1, 3)
        )  # (pd, ne, 128, 128)

        xpT = np.asarray(p["x_proj_w"], np.float32).T  # (din, R+32)
        t[f"xpw{l}"] = np.ascontiguousarray(xpT.reshape(nb, 128, R + 32)).astype(NPBF16)

        t[f"dtpw{l}"] = np.ascontiguousarray(
            np.asarray(p["dt_proj_w"], np.float32).T
        )  # (R, din)

        opT = np.asarray(p["out_proj_w"], np.float32).T  # (din, d)
        t[f"opw{l}"] = np.ascontiguousarray(
            opT.reshape(nb, 128, pd, 128).transpose(0, 2, 1, 3)
        ).astype(NPBF16)  # (nb, pd, 128, 128)

        def packv(x, nblk):
            return np.ascontiguousarray(
                np.asarray(x, np.float32).reshape(nblk, 128).T
            )

        t[f"bconv{l}"] = packv(bconv, pd)
        t[f"dwb{l}"] = packv(p["dwconv_b"], nb)
        t[f"dtb{l}"] = packv(p["dt_proj_b"], nb)
        t[f"Dv{l}"] = packv(p["D"], nb)
        t[f"gam{l}"] = packv(p["ln_gamma"], pd)
        t[f"bet{l}"] = packv(p["ln_beta"], pd)
        t[f"dww{l}"] = np.ascontiguousarray(
            np.asarray(p["dwconv_w"], np.float32).reshape(nb, 128, 4).transpose(1, 0, 2)
        )  # (128, nb, 4)

        A = -np.exp(np.asarray(p["A_log"], np.float32))  # (din, 16)
        if np.allclose(A, A[0:1, :], rtol=0, atol=0):
            a_ns.append([float(x) for x in A[0]])
        else:
            a_ns.append(None)
        t[f"at{l}"] = np.ascontiguousarray(
            A.reshape(nb, 128, 16).transpose(1, 0, 2).reshape(128, nb * 16)
        )
    return t, a_ns


# --------------------------------------------------------------------------
# device kernel
# --------------------------------------------------------------------------
def build_kernel(tc, I, out_ap, a_ns, use_silu=True):
    """use_silu=False decomposes SiLU into Sigmoid+mult (CoreSim lacks Silu)."""
    nc = tc.nc
    with ExitStack() as ctx:
        wp = ctx.enter_context(tc.tile_pool(name="wp", bufs=2))
        ap_ = ctx.enter_context(tc.tile_pool(name="ap", bufs=1))
        sp = ctx.enter_context(tc.tile_pool(name="sp", bufs=2))
        pp = ctx.enter_context(tc.tile_pool(name="pp", bufs=2, space="PSUM"))
        dr = ctx.enter_context(tc.tile_pool(name="dr", bufs=2, space="DRAM"))

        ones16 = ap_.tile([128, 1], BF16, tag="ones")
        nc.vector.memset(ones16, 1.0)
        eps_t = ap_.tile([1, 1], F32, tag="epsln")
        nc.vector.memset(eps_t, EPS_LN)

        xin = None
        for l, m in enumerate(LAYERS):
            prev, d, din, R = m["prev"], m["d"], m["din"], m["R"]
            pk, pd, nb, ne = prev // 128, d // 128, din // 128, 2 * din // 128
            RC = R + 32

            # ---- load per-layer small tensors ----
            bconv = ap_.tile([128, pd], F32, tag="bconv", bufs=1)
            nc.sync.dma_start(out=bconv, in_=I[f"bconv{l}"])
            dwb = ap_.tile([128, nb], F32, tag="dwb", bufs=1)
            nc.sync.dma_start(out=dwb, in_=I[f"dwb{l}"])
            dtb = ap_.tile([128, nb], F32, tag="dtb", bufs=1)
            nc.sync.dma_start(out=dtb, in_=I[f"dtb{l}"])
            Dv = ap_.tile([128, nb], F32, tag="Dv", bufs=1)
            nc.sync.dma_start(out=Dv, in_=I[f"Dv{l}"])
            gam = ap_.tile([128, pd], F32, tag="gam", bufs=1)
            nc.sync.dma_start(out=gam, in_=I[f"gam{l}"])
            bet = ap_.tile([128, pd], F32, tag="bet", bufs=1)
            nc.sync.dma_start(out=bet, in_=I[f"bet{l}"])
            dww = ap_.tile([128, nb, 4], F32, tag="dww", bufs=1)
            nc.sync.dma_start(out=dww, in_=I[f"dww{l}"])
            xpw = wp.tile([128, nb, RC], BF16, tag="wxp", bufs=1)
            nc.sync.dma_start(
                out=xpw, in_=I[f"xpw{l}"].rearrange("k c f -> c k f")
            )
            dtpw = wp.tile([R, din], F32, tag="wdtp", bufs=1)
            nc.sync.dma_start(out=dtpw, in_=I[f"dtpw{l}"])
            if a_ns[l] is None:
                a_t = ap_.tile([128, nb * 16], F32, tag="at", bufs=1)
                nc.sync.dma_start(out=a_t, in_=I[f"at{l}"])

            # ---- layer 0 input ----
            if l == 0:
                xin = ap_.tile([128, pk, 514], F32, tag="xio0")
                nc.sync.dma_start(
                    out=xin, in_=I["x_t"].rearrange("k c t -> c k t")
                )

            # ================= CNN =================
            cnn = ap_.tile([128, pd, 514], F32, tag="cnn")
            nc.vector.memset(cnn[:, :, 0:1], 0.0)
            nc.vector.memset(cnn[:, :, 513:514], 0.0)
            for ot in range(pd):
                ps = pp.tile([128, 512], F32, tag="ps", bufs=3)
                nmm = pk * 3
                i = 0
                for kt in range(pk):
                    for tap in range(3):
                        sl = wp.tile([128, 128], F32, tag="wslab", bufs=8)
                        nc.sync.dma_start(out=sl, in_=I[f"convw{l}"][kt, tap, ot])
                        nc.tensor.matmul(
                            ps,
                            sl,
                            xin[:, kt, tap : tap + 512],
                            start=(i == 0),
                            stop=(i == nmm - 1),
                        )
                        i += 1
                if m["skip"]:
                    ps2 = pp.tile([128, 512], F32, tag="ps2", bufs=1)
                    for kt in range(pk):
                        sl = wp.tile([128, 128], F32, tag="wslab", bufs=8)
                        nc.sync.dma_start(out=sl, in_=I[f"skipw{l}"][kt, ot])
                        nc.tensor.matmul(
                            ps2,
                            sl,
                            xin[:, kt, 1:513],
                            start=(kt == 0),
                            stop=(kt == pk - 1),
                        )
                nc.scalar.activation(
                    out=cnn[:, ot, 1:513],
                    in_=ps,
                    func=AF.Relu,
                    bias=bconv[:, ot : ot + 1],
                    scale=1.0,
                )
                if m["skip"]:
                    nc.vector.tensor_tensor(
                        out=cnn[:, ot, 1:513],
                        in0=cnn[:, ot, 1:513],
                        in1=ps2,
                        op=OP.add,
                    )
                else:
                    nc.vector.tensor_tensor(
                        out=cnn[:, ot, 1:513],
                        in0=cnn[:, ot, 1:513],
                        in1=xin[:, ot, 1:513],
                        op=OP.add,
                    )

            # ================= in_proj =================
            xm = ap_.tile([128, nb, 515], BF16, tag="xm")
            nc.vector.memset(xm[:, :, 0:3], 0.0)
            zs = ap_.tile([128, nb, 512], BF16, tag="zs")
            for et in range(ne):
                ps = pp.tile([128, 512], F32, tag="ps", bufs=3)
                for kt in range(pd):
                    sl = wp.tile([128, 128], F32, tag="wslab", bufs=8)
                    nc.sync.dma_start(out=sl, in_=I[f"inpw{l}"][kt, et])
                    nc.tensor.matmul(
                        ps,
                        sl,
                        cnn[:, kt, 1:513],
                        start=(kt == 0),
                        stop=(kt == pd - 1),
                    )
                if et < nb:
                    nc.scalar.activation(
                        out=xm[:, et, 3:515], in_=ps, func=AF.Copy
                    )
                elif use_silu:
                    nc.scalar.activation(
                        out=zs[:, et - nb, :], in_=ps, func=AF.Silu
                    )
                else:
                    zsg = sp.tile([128, 512], BF16, tag="zsg", bufs=2)
                    nc.scalar.activation(out=zsg, in_=ps, func=AF.Sigmoid)
                    zrw = sp.tile([128, 512], BF16, tag="zrw", bufs=2)
                    nc.scalar.activation(out=zrw, in_=ps, func=AF.Copy)
                    nc.vector.tensor_tensor(
                        out=zs[:, et - nb, :], in0=zsg, in1=zrw, op=OP.mult
                    )

            # ================= depthwise causal conv + SiLU =================
            xconv = ap_.tile([128, nb, 512], BF16, tag="xconv")
            for b in range(nb):
                dca = sp.tile([128, 512], BF16, tag="dca", bufs=2)
                nc.vector.tensor_scalar(
                    out=dca,
                    in0=xm[:, b, 0:512],
                    scalar1=dww[:, b, 0:1],
                    scalar2=None,
                    op0=OP.mult,
                )
                for k in range(1, 4):
                    nc.vector.scalar_tensor_tensor(
                        out=dca,
                        in0=xm[:, b, k : k + 512],
                        scalar=dww[:, b, k : k + 1],
                        in1=dca,
                        op0=OP.mult,
                        op1=OP.add,
                    )
                if use_silu:
                    nc.scalar.activation(
                        out=xconv[:, b, :],
                        in_=dca,
                        func=AF.Silu,
                        bias=dwb[:, b : b + 1],
                        scale=1.0,
                    )
                else:
                    nc.vector.tensor_scalar(
                        out=dca,
                        in0=dca,
                        scalar1=dwb[:, b : b + 1],
                        scalar2=None,
                        op0=OP.add,
                    )
                    xsg = sp.tile([128, 512], BF16, tag="xsg", bufs=2)
                    nc.scalar.activation(out=xsg, in_=dca, func=AF.Sigmoid)
                    nc.vector.tensor_tensor(
                        out=xconv[:, b, :], in0=dca, in1=xsg, op=OP.mult
                    )

            # ================= x_proj -> dbc =================
            dbcps = pp.tile([RC, 512], F32, tag="dbcps", bufs=1)
            for kt in range(nb):
                nc.tensor.matmul(
                    dbcps,
                    xpw[:, kt, :],
                    xconv[:, kt, :],
                    start=(kt == 0),
                    stop=(kt == nb - 1),
                )
            dbc = ap_.tile([RC, 512], F32, tag="dbc", bufs=1)
            nc.scalar.activation(out=dbc, in_=dbcps, func=AF.Copy)
            dbc16 = ap_.tile([RC, 512], BF16, tag="dbc16", bufs=1)
            nc.scalar.activation(out=dbc16, in_=dbcps, func=AF.Copy)
            bc_scr = dr.tile([32, 512], BF16, tag="bcscr")
            nc.sync.dma_start(out=bc_scr, in_=dbc16[R:RC, :])

            # ================= dt_proj -> softplus (ln(exp(x)+1)) ========
            dt = ap_.tile([128, nb, SEP], F32, tag="dt")
            nc.vector.memset(dt[:, :, 0:1], 1e8)
            for b in range(nb):
                ps = pp.tile([128, 512], F32, tag="ps", bufs=3)
                nc.tensor.matmul(
                    ps,
                    dtpw[:, b * 128 : (b + 1) * 128],
                    dbc[0:R, :],
                    start=True,
                    stop=True,
                )
                nc.scalar.activation(
                    out=dt[:, b, 1:513],
                    in_=ps,
                    func=AF.Exp,
                    bias=dtb[:, b : b + 1],
                    scale=1.0,
                )
                nc.scalar.activation(
                    out=dt[:, b, 1:513],
                    in_=dt[:, b, 1:513],
                    func=AF.Ln,
                    bias=1.0,
                )

            # u = dt * xconv (bf16)
            u = ap_.tile([128, nb, SEP], BF16, tag="u")
            nc.vector.tensor_tensor(
                out=u[:, :, 1:513],
                in0=dt[:, :, 1:513],
                in1=xconv,
                op=OP.mult,
            )

            # ================= selective scan =================
            yacc = ap_.tile([128, nb, 512], BF16, tag="yacc")
            dt2d = dt.rearrange("p a b -> p (a b)")
            for n in range(16):
                da = sp.tile([128, nb, SEP], BF16, tag="da", bufs=2)
                if a_ns[l] is not None:
                    nc.scalar.activation(
                        out=da.rearrange("p a b -> p (a b)"),
                        in_=dt2d,
                        func=AF.Exp,
                        scale=float(a_ns[l][n]),
                    )
                else:
                    for b in range(nb):
                        nc.scalar.activation(
                            out=da[:, b, :],
                            in_=dt[:, b, :],
                            func=AF.Exp,
                            scale=a_t[:, b * 16 + n : b * 16 + n + 1],
                        )
                dbx = sp.tile([128, nb, SEP], BF16, tag="dbx", bufs=2)
                nc.vector.memset(dbx[:, :, 0:1], 0.0)
                Bn = sp.tile([128, 512], BF16, tag="bn", bufs=2)
                nc.sync.dma_start(
                    out=Bn, in_=bc_scr[n : n + 1, :].partition_broadcast(128)
                )
                nc.vector.tensor_tensor(
                    out=dbx[:, :, 1:513],
                    in0=u[:, :, 1:513],
                    in1=Bn.unsqueeze(1).to_broadcast([128, nb, 512]),
                    op=OP.mult,
                )
                h = sp.tile([128, nb, SEP], BF16, tag="h", bufs=2)
                nc.vector.tensor_tensor_scan(
                    out=h.rearrange("p a b -> p (a b)"),
                    data0=da.rearrange("p a b -> p (a b)"),
                    data1=dbx.rearrange("p a b -> p (a b)"),
                    initial=0.0,
                    op0=OP.mult,
                    op1=OP.add,
                )
                Cn = sp.tile([128, 512], BF16, tag="cn", bufs=2)
                nc.sync.dma_start(
                    out=Cn, in_=bc_scr[16 + n : 17 + n, :].partition_broadcast(128)
                )
                tmp = sp.tile([128, nb, 512], BF16, tag="tmp", bufs=2)
                nc.vector.tensor_tensor(
                    out=tmp,
                    in0=h[:, :, 1:513],
                    in1=Cn.unsqueeze(1).to_broadcast([128, nb, 512]),
                    op=OP.mult,
                )
                if n == 0:
                    nc.vector.tensor_copy(yacc, tmp)
                else:
                    nc.vector.tensor_tensor(out=yacc, in0=yacc, in1=tmp, op=OP.add)

            # ================= y = (yacc + xconv*D) * silu(z) ============
            ym = ap_.tile([128, nb, 512], BF16, tag="ym")
            for b in range(nb):
                nc.vector.scalar_tensor_tensor(
                    out=ym[:, b, :],
                    in0=xconv[:, b, :],
                    scalar=Dv[:, b : b + 1],
                    in1=yacc[:, b, :],
                    op0=OP.mult,
                    op1=OP.add,
                )
            nc.vector.tensor_tensor(out=ym, in0=ym, in1=zs, op=OP.mult)

            # ================= out_proj + LN stats =================
            mo = ap_.tile([128, pd, 512], BF16, tag="mo")
            s1ps = pp.tile([1, 512], F32, tag="s1ps", bufs=1)
            s2ps = pp.tile([1, 512], F32, tag="s2ps", bufs=1)
            for ot in range(pd):
                ps = pp.tile([128, 512], F32, tag="ps", bufs=3)
                for kt in range(nb):
                    sl = wp.tile([128, 128], BF16, tag="wslab16", bufs=8)
                    nc.sync.dma_start(out=sl, in_=I[f"opw{l}"][kt, ot])
                    nc.tensor.matmul(
                        ps,
                        sl,
                        ym[:, kt, :],
                        start=(kt == 0),
                        stop=(kt == nb - 1),
                    )
                nc.scalar.activation(out=mo[:, ot, :], in_=ps, func=AF.Copy)
                mosq = sp.tile([128, 512], BF16, tag="mosq", bufs=2)
                nc.scalar.activation(out=mosq, in_=ps, func=AF.Square)
                nc.tensor.matmul(
                    s1ps,
                    ones16,
                    mo[:, ot, :],
                    start=(ot == 0),
                    stop=(ot == pd - 1),
                )
                nc.tensor.matmul(
                    s2ps,
                    ones16,
                    mosq,
                    start=(ot == 0),
                    stop=(ot == pd - 1),
                )

            # ---- LN scalars on (1, 512) ----
            mean = sp.tile([1, 512], F32, tag="lnmean", bufs=1)
            nc.scalar.activation(out=mean, in_=s1ps, func=AF.Copy, scale=1.0 / d)
            ex2 = sp.tile([1, 512], F32, tag="lnex2", bufs=1)
            nc.scalar.activation(out=ex2, in_=s2ps, func=AF.Copy, scale=1.0 / d)
            var = sp.tile([1, 512], F32, tag="lnvar", bufs=1)
            nc.vector.tensor_tensor(out=var, in0=mean, in1=mean, op=OP.mult)
            nc.vector.tensor_tensor(out=var, in0=ex2, in1=var, op=OP.subtract)
            # rstd = exp(-0.5 * ln(var + eps))
            nc.scalar.activation(out=var, in_=var, func=AF.Ln, bias=eps_t[0:1, 0:1])
            mean16 = sp.tile([1, 512], BF16, tag="lnm16", bufs=1)
            nc.scalar.activation(out=mean16, in_=mean, func=AF.Copy)
            rstd16 = sp.tile([1, 512], BF16, tag="lnr16", bufs=1)
            nc.scalar.activation(out=rstd16, in_=var, func=AF.Exp, scale=-0.5)
            mr_scr = dr.tile([2, 512], BF16, tag="mrscr")
            nc.sync.dma_start(out=mr_scr[0:1, :], in_=mean16)
            nc.sync.dma_start(out=mr_scr[1:2, :], in_=rstd16)
            mean_b = sp.tile([128, 512], BF16, tag="meanb", bufs=1)
            nc.sync.dma_start(
                out=mean_b, in_=mr_scr[0:1, :].partition_broadcast(128)
            )
            rstd_b = sp.tile([128, 512], BF16, tag="rstdb", bufs=1)
            nc.sync.dma_start(
                out=rstd_b, in_=mr_scr[1:2, :].partition_broadcast(128)
            )

            # ---- normalize + affine + residual -> next layer input ----
            last = l == len(LAYERS) - 1
            xout = ap_.tile([128, pd, 514], F32, tag=f"xio{(l + 1) % 2}")
            if not last:
                nc.vector.memset(xout[:, :, 0:1], 0.0)
                nc.vector.memset(xout[:, :, 513:514], 0.0)
            for ot in range(pd):
                xn = sp.tile([128, 512], BF16, tag="xn", bufs=2)
                nc.vector.tensor_tensor(
                    out=xn, in0=mo[:, ot, :], in1=mean_b, op=OP.subtract
                )
                nc.vector.tensor_tensor(out=xn, in0=xn, in1=rstd_b, op=OP.mult)
                nc.scalar.activation(
                    out=xout[:, ot, 1:513],
                    in_=xn,
                    func=AF.Identity,
                    bias=bet[:, ot : ot + 1],
                    scale=gam[:, ot : ot + 1],
                )
                nc.vector.tensor_tensor(
                    out=xout[:, ot, 1:513],
                    in0=xout[:, ot, 1:513],
                    in1=cnn[:, ot, 1:513],
                    op=OP.add,
                )
                if last:
                    nc.sync.dma_start(out=out_ap[ot], in_=xout[:, ot, 1:513])
            xin = xout


# --------------------------------------------------------------------------
# execution: cached PJRT runner (no donation so buffers are reusable)
# --------------------------------------------------------------------------
def _get_runner(nc):
    key = ("runner", id(nc))
    if key in _cache:
        return _cache[key]
    import jax
    from jax.experimental.shard_map import shard_map
    from jax.sharding import Mesh, NamedSharding, PartitionSpec

    from concourse import bass2jax

    bass2jax.install_neuronx_cc_hook()

    partition_name = (
        nc.partition_id_tensor.name if nc.partition_id_tensor else None
    )
    in_names, out_names, out_avals, zero_outs = [], [], [], []
    for alloc in nc.m.functions[0].allocations:
        if not isinstance(alloc, mybir.MemoryLocationSet):
            continue
        name = alloc.memorylocations[0].name
        if alloc.kind == "ExternalInput":
            if name != partition_name:
                in_names.append(name)
        elif alloc.kind == "ExternalOutput":
            shape = tuple(alloc.tensor_shape)
            dtype = mybir.dt.np(alloc.dtype)
            out_names.append(name)
            out_avals.append(jax.core.ShapedArray(shape, dtype))
            zero_outs.append(np.zeros(shape, dtype))
    n_params = len(in_names)
    all_in_names = list(in_names) + list(out_names)
    if partition_name is not None:
        all_in_names.append(partition_name)

    def _body(*args):
        operands = list(args)
        if partition_name is not None:
            operands.append(bass2jax.partition_id_tensor())
        outs = bass2jax._bass_exec_p.bind(
            *operands,
            out_avals=tuple(out_avals),
            in_names=tuple(all_in_names),
            out_names=tuple(out_names),
            lowering_input_output_aliases=(),
            sim_require_finite=True,
            sim_require_nnan=True,
            nc=nc,
        )
        return tuple(outs)

    devices = jax.devices()[:N_CORES]
    mesh = Mesh(np.asarray(devices), ("core",))
    n_args = n_params + len(out_names)
    sharded = jax.jit(
        shard_map(
            _body,
            mesh=mesh,
            in_specs=(PartitionSpec("core"),) * n_args,
            out_specs=(PartitionSpec("core"),) * len(out_names),
            check_rep=False,
        ),
        keep_unused=True,
    )
    sharding = NamedSharding(mesh, PartitionSpec("core"))
    runner = dict(
        jit=sharded,
        in_names=in_names,
        out_names=out_names,
        out_avals=out_avals,
        zero_outs=zero_outs,
        n_params=n_params,
        sharding=sharding,
    )
    _cache[key] = runner
    return runner


def _run_spmd(nc, in_maps):
    import jax

    r = _get_runner(nc)
    n = len(in_maps)
    concat_in = [
        np.concatenate([np.asarray(in_maps[c][name]) for c in range(n)], axis=0)
        for name in r["in_names"]
    ]
    concat_zeros = [
        np.zeros((n * z.shape[0], *z.shape[1:]), z.dtype) for z in r["zero_outs"]
    ]
    dev_in = [jax.device_put(a, r["sharding"]) for a in concat_in]
    dev_zero = [jax.device_put(a, r["sharding"]) for a in concat_zeros]
    out_arrs = r["jit"](*dev_in, *dev_zero)
    jax.block_until_ready(out_arrs)
    last_run_info["dev_args"] = (dev_in, dev_zero)
    last_run_info["runner"] = r
    return [
        {
            name: np.asarray(out_arrs[i]).reshape(n, *r["out_avals"][i].shape)[c]
            for i, name in enumerate(r["out_names"])
        }
        for c in range(n)
    ]


def benchmark(iters=20):
    """Re-run the last compiled kernel on the resident device inputs."""
    import time

    import jax

    r = last_run_info["runner"]
    dev_in, dev_zero = last_run_info["dev_args"]
    times = []
    for _ in range(iters):
        t0 = time.perf_counter()
        out = r["jit"](*dev_in, *dev_zero)
        jax.block_until_ready(out)
        times.append(time.perf_counter() - t0)
    return times


# --------------------------------------------------------------------------
# entry point
# --------------------------------------------------------------------------
def _build(tensors, a_ns, x_shape_key):
    key = ("nc", x_shape_key, tuple(a is not None for a in a_ns))
    if key in _cache:
        return _cache[key]
    nc = bacc.Bacc(
        "TRN2", target_bir_lowering=False, debug=False, num_devices=N_CORES
    )
    I = {}
    for name, arr in tensors.items():
        dt_ = BF16 if arr.dtype == NPBF16 else F32
        I[name] = nc.dram_tensor(name, arr.shape, dt_, kind="ExternalInput").ap()
    I["x_t"] = nc.dram_tensor("x_t", (2, 128, 514), F32, kind="ExternalInput").ap()
    out_ap = nc.dram_tensor("out", (4, 128, 512), F32, kind="ExternalOutput").ap()
    with tile.TileContext(nc) as tc:
        build_kernel(tc, I, out_ap, a_ns)
    nc.compile()
    _cache[key] = (nc, I, out_ap)
    return nc, I, out_ap


def kernel(x, params, _trace=False):
    x = np.asarray(x, np.float32)
    assert x.shape == (B, L, 256), x.shape
    tensors, a_ns = _prep_weights(params)
    nc, _, _ = _build(tensors, a_ns, x.shape)

    in_maps = []
    for b in range(B):
        im = dict(tensors)
        xt = np.zeros((2, 128, 514), np.float32)
        xt[:, :, 1:513] = x[b].T.reshape(2, 128, 512)
        im["x_t"] = xt
        in_maps.append(im)

    results = _run_spmd(nc, in_maps)

    outs = []
    for b in range(B):
        oc = np.asarray(results[b]["out"], np.float32)  # (4, 128, 512)
        outs.append(oc.reshape(512, 512).T)  # (t, ch)
    return np.stack(outs).astype(np.float32)
